# revision 28
# baseline (speedup 1.0000x reference)
"""GQA attention (16 q heads / 4 kv heads, HD=128, S=4096, D=2048) with RoPE,
causal mask, log-gate on kv positions, softmax, and output projection —
distributed over 8 NeuronCores.

Sharding: head-parallel. Core c computes q heads {2c, 2c+1} and kv head c//2.
Wq/Wk/Wv are split column-wise, Wo row-wise; each core produces a partial
[S, D] output (its 2 heads' contribution through Wo) and the host sums the 8
partials (the unshard step of the row-parallel Wo matmul).

On-device layout strategy (v3):
 - All matmul inputs fp16 (1 cycle/row like bf16, 8x the mantissa).
 - Projections computed transposed: qT/kT = W.T @ X.T with d on partitions.
 - Head-dim of q/k is PERMUTED (d and d+64 placed 16 apart within the same
   32-partition quadrant) so the RoPE rotate-half is a single DVE
   stream_shuffle straight out of PSUM — no SBUF round-trip, no DMA.
   Scores are invariant to any consistent d-permutation of q and k.
 - All DRAM inputs host-staged in exact SBUF tile layout: every DMA is 128
   large contiguous per-partition descriptors (the v2 kernel was
   descriptor-latency-bound with 41K small descriptors).
 - Attention computed transposed (scores^T [j, i]); max-free softmax with a
   constant -8 shift so fp16 exp/accumulate never overflows; log-gate +
   shift folded into the exp as a per-partition activation bias.
 - Causality structural: upper-triangle blocks skipped, fully-masked query
   columns of diagonal blocks skipped (ragged matmuls), the diagonal 128
   queries masked by a 0/1 fp16 multiply after exp.
 - Softmax denominators: fp16 DVE accumulation of exp tiles, one M=1
   ones-matmul per head, reciprocal_approx_fast, gpsimd partition-broadcast.
 - Emission pipelined: attn(nb) -> sums -> proj(nb+1) -> normalize(nb) ->
   outproj(nb); x(nb+1) prefetched an iteration ahead.
 - Per-core partial outputs written fp16 (halves write traffic; host sums
   in fp32).
"""

import math
from contextlib import ExitStack

import numpy as np

import concourse.bass as bass
import concourse.mybir as mybir
import concourse.tile as tile
from concourse import bacc
from concourse._compat import with_exitstack
from concourse.bass import ds
from concourse.bass_utils import run_bass_kernel_spmd
from concourse.masks import make_identity

P = 128
F = 512            # free-dim chunk (one PSUM bank of fp32)
S = 4096
D = 2048
HD = 128
KO = D // P        # 16 k-chunks for the projections
NB = S // F        # 8 sequence chunks
NJB = S // P       # 32 key blocks
F32 = mybir.dt.float32
FP16 = mybir.dt.float16
SHIFT = 8.0        # constant softmax shift (cancels in the ratio)
# quadrant-local swap of the 16-element halves: partition p <-> p +- 16
SHUF = list(range(16, 32)) + list(range(16))
# PERM[p] = head-dim stored at partition p (pairs d, d+64 live 16 apart)
PERM = np.array(
    [16 * q + s if s < 16 else 64 + 16 * q + (s - 16)
     for q in range(4) for s in range(32)]
)


@with_exitstack
def _body(ctx: ExitStack, tc: tile.TileContext, io: dict):
    nc = tc.nc

    persist = ctx.enter_context(tc.tile_pool(name="persist", bufs=1))
    qT = persist.tile([P, 2, S], FP16, tag="qT")        # [d, h, i]
    kT = persist.tile([P, S], FP16, tag="kT")           # [d, j]
    vv = persist.tile([P, NJB, HD], FP16, tag="vv")     # [j, jb, d]
    attnT = persist.tile([P, 2, S], FP16, tag="attnT")  # [d, h, i] normalized
    logg = persist.tile([P, NJB], F32, tag="logg")      # log(gate)-SHIFT, [j, jb]
    dmask01 = persist.tile([P, P], FP16, tag="dmask01")
    ident = persist.tile([P, P], F32, tag="ident")
    ones16 = persist.tile([P, 1], FP16, tag="ones16")

    wpool = ctx.enter_context(tc.tile_pool(name="wpool", bufs=1))
    wq = wpool.tile([P, KO, 2 * HD], FP16, tag="wq")
    wk = wpool.tile([P, KO, HD], FP16, tag="wk")
    wv = wpool.tile([P, KO, HD], FP16, tag="wv")
    wo = wpool.tile([P, 2, D], FP16, tag="wo")

    xt_pool = ctx.enter_context(tc.tile_pool(name="xt", bufs=2))
    tab_pool = ctx.enter_context(tc.tile_pool(name="tab", bufs=2))
    rope_pool = ctx.enter_context(tc.tile_pool(name="rope", bufs=2))
    exp_pool = ctx.enter_context(tc.tile_pool(name="exp", bufs=4))
    acc_pool = ctx.enter_context(tc.tile_pool(name="acc", bufs=2))
    bc_pool = ctx.enter_context(tc.tile_pool(name="bc", bufs=2))
    ob_pool = ctx.enter_context(tc.tile_pool(name="ob", bufs=3))
    # PSUM budget (8 banks): psSc pair tiles 2x2 + psAV 3 + psSum 1 = 8.
    psSc = ctx.enter_context(tc.tile_pool(name="psSc", bufs=2, space="PSUM"))
    psAV = ctx.enter_context(tc.tile_pool(name="psAV", bufs=3, space="PSUM"))
    psSum = ctx.enter_context(tc.tile_pool(name="psSum", bufs=1, space="PSUM"))

    def load_x(nb):
        xtile = xt_pool.tile([P, KO, F], FP16, tag="xt")
        nc.sync.dma_start(xtile[:], io["xs"][nb])
        tabs = tab_pool.tile([P, 4, F], FP16, tag="tabs")
        nc.sync.dma_start(tabs[:], io["tabs"][nb])
        return xtile, tabs

    # Startup: x + q-weights first so the first proj chain starts while the
    # remaining weights/constants stream in.
    xq0 = xt_pool.tile([P, KO, F], FP16, tag="xt", name="xq0")
    nc.sync.dma_start(xq0[:, 0:2, :], io["xs"][0, :, 0:2, :])
    nc.sync.dma_start(wq[:, 0:2, :], io["wq"][:, 0:2, :])
    nc.sync.dma_start(xq0[:, 2:8, :], io["xs"][0, :, 2:8, :])
    nc.sync.dma_start(wq[:, 2:8, :], io["wq"][:, 2:8, :])
    nc.sync.dma_start(xq0[:, 8:16, :], io["xs"][0, :, 8:16, :])
    nc.sync.dma_start(wq[:, 8:16, :], io["wq"][:, 8:16, :])
    tabs0 = tab_pool.tile([P, 4, F], FP16, tag="tabs", name="tabs0")
    nc.sync.dma_start(tabs0[:], io["tabs"][0])
    nc.sync.dma_start(wk[:], io["wk"])
    nc.sync.dma_start(wv[:], io["wv"])
    nc.sync.dma_start(logg[:], io["logg"])
    nc.sync.dma_start(dmask01[:], io["dmask01"])
    nc.sync.dma_start(wo[:], io["wo"])
    make_identity(nc, ident[:])
    nc.vector.memset(ones16[:], 1.0)

    def rope(ps, ct, st, dest, pair=True):
        src_ap = ps[:, 0, :] if pair else ps[:]
        rot = rope_pool.tile([P, F], F32, tag="rot", name="rot")
        nc.vector.stream_shuffle(rot[:], src_ap, mask=SHUF)
        t1 = rope_pool.tile([P, F], F32, tag="t1", name="t1")
        nc.vector.tensor_tensor(t1[:], src_ap, ct, op=mybir.AluOpType.mult)
        r2 = rope_pool.tile([P, F], F32, tag="r2", name="r2")
        nc.vector.tensor_tensor(r2[:], rot[:], st, op=mybir.AluOpType.mult)
        nc.vector.tensor_tensor(dest, t1[:], r2[:], op=mybir.AluOpType.add)

    def proj(nb, xtile, tabs, with_k=True, mid=None):
        """Projections + rope + v-transpose for sequence chunk nb. `mid` is
        emitted after the first chain so the tensor engine has ready work
        (the Q0 chain) to absorb the wait for the attn accumulator."""
        sl = ds(nb * F, F)

        def chain(w_sb, m0):
            ps = psSc.tile([P, 2, F], F32, tag="sc")
            for ko in range(KO):
                nc.tensor.matmul(
                    ps[:, 0, :],
                    lhsT=w_sb[:, ko, ds(m0, P)],
                    rhs=xtile[:, ko, :],
                    start=(ko == 0),
                    stop=(ko == KO - 1),
                )
            return ps

        rope(chain(wq, 0), tabs[:, 0, :], tabs[:, 1, :], qT[:, 0, sl])
        mids = [mid() if mid is not None else None]
        rope(chain(wq, P), tabs[:, 0, :], tabs[:, 1, :], qT[:, 1, sl])
        if with_k:
            rope(chain(wk, 0), tabs[:, 2, :], tabs[:, 3, :], kT[:, sl])

        psv = chain(wv, 0)
        vT = rope_pool.tile([P, F], F32, tag="vT")
        nc.scalar.copy(vT[:], psv[:, 0, :])
        pt = psSc.tile([P, 2, F], F32, tag="sc")
        for isub in range(4):
            nc.tensor.transpose(
                pt[:, 0, ds(isub * P, P)], vT[:, ds(isub * P, P)], ident[:]
            )
        nc.scalar.copy(vv[:, ds(nb * 4, 4), :], pt[:, 0, :])

    def attn(nb, kchain=None):
        """Attention for q-chunk nb (both heads); returns psum AV tiles +
        the fp16 exp-accumulator (for the sum collapse). If kchain is
        (xtile, tabs, nbn), the K projection chain for chunk nbn is
        interleaved into the jb loop (fills the tensor engine's slack in
        the scalar-paced attention window, psSum bank is free here)."""
        njb = 4 * nb + 4
        avs = [
            psAV.tile([P, F], F32, tag="av", name=f"av{h}") for h in range(2)
        ]
        acc = acc_pool.tile([P, 2, F], FP16, tag="acc")
        ck_done = 0
        if kchain is not None:
            xtn, tabn, nbn = kchain
            ck = psSum.tile([P, F], F32, tag="cksum", name="ck")

            def ck_emit(limit):
                nonlocal ck_done
                while ck_done < min(limit, KO):
                    nc.tensor.matmul(
                        ck[:],
                        lhsT=wk[:, ck_done, :],
                        rhs=xtn[:, ck_done, :],
                        start=(ck_done == 0),
                        stop=(ck_done == KO - 1),
                    )
                    ck_done += 1


        for jb in range(njb):
            dp = jb - 4 * nb
            # Queries below the diagonal block are fully masked: skip them.
            q0 = dp * P if dp > 0 else 0
            n = F - q0
            sc = psSc.tile([P, 2, F], F32, tag="sc")
            for h in range(2):
                nc.tensor.matmul(
                    sc[:, h, 0:n],
                    lhsT=kT[:, ds(jb * P, P)],
                    rhs=qT[:, h, ds(nb * F + q0, n)],
                    start=True,
                    stop=True,
                )
            ex = exp_pool.tile([P, 2, F], FP16, tag="ex")
            nc.scalar.activation(
                ex[:, :, 0:n], sc[:, :, 0:n],
                mybir.ActivationFunctionType.Exp,
                bias=logg[:, jb : jb + 1],
            )
            if dp >= 0:
                # within-block triangle mask on the diagonal 128 queries
                for h in range(2):
                    nc.vector.tensor_tensor(
                        ex[:, h, 0:P], ex[:, h, 0:P], dmask01[:],
                        op=mybir.AluOpType.mult,
                    )
            if jb == 0:
                nc.vector.tensor_copy(acc[:], ex[:])
            else:
                nc.vector.tensor_tensor(
                    acc[:, :, q0:F], acc[:, :, q0:F], ex[:, :, 0:n],
                    op=mybir.AluOpType.add,
                )
            for h in range(2):
                nc.tensor.matmul(
                    avs[h][:, q0:F],
                    lhsT=vv[:, jb, :],
                    rhs=ex[:, h, 0:n],
                    start=(jb == 0),
                    stop=(jb == njb - 1),
                )
            if kchain is not None and jb >= 6:
                # uniform spread of the 16 chain MMs over the eligible jb
                ck_emit((jb - 5) * KO // max(njb - 6, 1))
        if kchain is not None:
            ck_emit(KO)
            rope(ck, tabn[:, 2, :], tabn[:, 3, :], kT[:, ds(nbn * F, F)],
                 pair=False)
        return avs, acc

    def sums_collapse(acc):
        sums = psSum.tile([P, F], F32, tag="cksum", name="sums")
        for h in range(2):
            nc.tensor.matmul(
                sums[ds(32 * h, 1), :],
                lhsT=ones16[:, 0:1],
                rhs=acc[:, h, :],
                start=True,
                stop=True,
            )
        return sums

    def normalize(nb, avs, sums):
        sl = ds(nb * F, F)
        for h in range(2):
            srow = bc_pool.tile([1, F], F32, tag=f"srow{h}", name=f"srow{h}")
            nc.scalar.copy(srow[:], sums[ds(32 * h, 1), :])
            rrow = bc_pool.tile([1, F], F32, tag=f"rrow{h}", name=f"rrow{h}")
            nc.vector.reciprocal_approx_fast(rrow[:], srow[:])
            rbc = bc_pool.tile([P, F], F32, tag=f"rbc{h}", name=f"rbc{h}")
            nc.gpsimd.partition_broadcast(rbc[:], rrow[0:1, :])
            nc.vector.tensor_tensor(
                attnT[:, h, sl], avs[h][:], rbc[:],
                op=mybir.AluOpType.mult,
            )

    def outproj(nb, last=False):
        for i4 in range(4):
            i2 = nb * 4 + i4
            split = last and i4 == 3
            ob = ob_pool.tile([P, D], FP16, tag="ob")
            for e in range(D // F):
                po = psAV.tile([P, F], F32, tag="av")
                for h in range(2):
                    nc.tensor.matmul(
                        po[:],
                        lhsT=attnT[:, h, ds(i2 * P, P)],
                        rhs=wo[:, h, ds(e * F, F)],
                        start=(h == 0),
                        stop=(h == 1),
                    )
                if e % 2 == 0:
                    nc.scalar.copy(ob[:, ds(e * F, F)], po[:])
                else:
                    nc.vector.tensor_copy(ob[:, ds(e * F, F)], po[:])
                if split:
                    nc.sync.dma_start(
                        io["outp"][ds(i2 * P, P), ds(e * F, F)],
                        ob[:, ds(e * F, F)],
                    )
            if not split:
                nc.sync.dma_start(io["outp"][ds(i2 * P, P), :], ob[:])

    # ---- pipelined emission ----
    # load_x(nb+2) is issued AFTER outproj(nb)'s output DMAs so the output
    # writes are never queued behind the multi-MB prefetch (head-of-line
    # blocking on the DMA queues was stalling psum-tile reuse).
    proj(0, xq0, tabs0)
    xqn, tabsn = load_x(1)
    for nb in range(NB):
        kc = (xqn, tabsn, nb + 1) if nb + 1 < NB else None
        avs, acc = attn(nb, kchain=kc)
        if nb + 1 < NB:
            xq_cur, tabs_cur = xqn, tabsn
            holder = []
            proj(nb + 1, xq_cur, tabs_cur, with_k=False,
                 mid=lambda: holder.append(sums_collapse(acc)))
            sums = holder[0]
        else:
            sums = sums_collapse(acc)
        normalize(nb, avs, sums)
        outproj(nb, last=(nb == NB - 1))
        if nb + 2 < NB:
            xqn, tabsn = load_x(nb + 2)


_NC_CACHE = None


def build_nc():
    global _NC_CACHE
    if _NC_CACHE is not None:
        return _NC_CACHE
    nc = bacc.Bacc("TRN2", target_bir_lowering=False, debug=False)
    io = {
        "xs": nc.dram_tensor("xs", [NB, P, KO, F], FP16,
                             kind="ExternalInput").ap(),
        "wq": nc.dram_tensor("wq", [P, KO, 2 * HD], FP16,
                             kind="ExternalInput").ap(),
        "wk": nc.dram_tensor("wk", [P, KO, HD], FP16,
                             kind="ExternalInput").ap(),
        "wv": nc.dram_tensor("wv", [P, KO, HD], FP16,
                             kind="ExternalInput").ap(),
        "wo": nc.dram_tensor("wo", [P, 2, D], FP16,
                             kind="ExternalInput").ap(),
        "tabs": nc.dram_tensor("tabs", [NB, P, 4, F], FP16,
                               kind="ExternalInput").ap(),
        "logg": nc.dram_tensor("logg", [P, NJB], F32,
                               kind="ExternalInput").ap(),
        "dmask01": nc.dram_tensor("dmask01", [P, P], FP16,
                                  kind="ExternalInput").ap(),
        "outp": nc.dram_tensor("outp", [S, D], FP16,
                               kind="ExternalOutput").ap(),
    }
    with tile.TileContext(nc) as tc:
        _body(tc, io)
    nc.compile()
    _NC_CACHE = nc
    return nc


def make_in_maps(hidden_states, attention_mask, cos, sin, gate, Wq, Wk, Wv, Wo):
    X = np.asarray(hidden_states, np.float32).reshape(S, D)
    # [nb, p, ko, s] = X[nb*F + s, ko*P + p]
    xs = np.ascontiguousarray(
        X.reshape(NB, F, KO, P).transpose(0, 3, 2, 1).astype(np.float16)
    )
    cosT = np.asarray(cos, np.float32).reshape(S, HD).T      # [d, s]
    sinT = np.asarray(sin, np.float32).reshape(S, HD).T
    cosP = cosT[PERM]
    sign = np.where(np.arange(P) % 32 < 16, np.float32(-1), np.float32(1))
    sinP = sign[:, None] * sinT[PERM]
    sc = np.float32(1.0 / math.sqrt(HD))
    tab = np.stack([cosP * sc, sinP * sc, cosP, sinP], axis=1)  # [128, 4, S]
    tabs = np.ascontiguousarray(
        tab.reshape(P, 4, NB, F).transpose(2, 0, 1, 3).astype(np.float16)
    )
    g = np.asarray(gate, np.float32).reshape(S) + np.float32(1e-8)
    logg = np.log(g).astype(np.float32) - np.float32(SHIFT)
    logg = np.ascontiguousarray(logg.reshape(NJB, P).T)
    jj = np.arange(P)[:, None]
    ii = np.arange(P)[None, :]
    dmask01 = np.ascontiguousarray(
        np.where(jj <= ii, 1, 0).astype(np.float16)
    )

    Wq = np.asarray(Wq, np.float32)
    Wk = np.asarray(Wk, np.float32)
    Wv = np.asarray(Wv, np.float32)
    Wo = np.asarray(Wo, np.float32)

    def stage_w(Wc, nheads, perm):
        # Wc [D, nheads*128] -> [p, ko, m] fp16 (m within-head order perm'd)
        if perm is not None:
            Wc = Wc.reshape(D, nheads, P)[:, :, perm].reshape(D, nheads * P)
        return np.ascontiguousarray(
            Wc.reshape(KO, P, nheads * P).transpose(1, 0, 2).astype(np.float16)
        )

    in_maps = []
    for c in range(8):
        g128 = c // 2
        wo_c = Wo[c * 256 : (c + 1) * 256, :]
        in_maps.append(
            {
                "xs": xs,
                "wq": stage_w(Wq[:, c * 256 : (c + 1) * 256], 2, PERM),
                "wk": stage_w(Wk[:, g128 * HD : (g128 + 1) * HD], 1, PERM),
                "wv": stage_w(Wv[:, g128 * HD : (g128 + 1) * HD], 1, None),
                "wo": np.ascontiguousarray(
                    wo_c.reshape(2, P, D).transpose(1, 0, 2).astype(np.float16)
                ),
                "tabs": tabs,
                "logg": logg,
                "dmask01": dmask01,
            }
        )
    return in_maps


def kernel(hidden_states, attention_mask, cos, sin, gate, Wq, Wk, Wv, Wo,
           **kwargs):
    nc = build_nc()
    in_maps = make_in_maps(
        hidden_states, attention_mask, cos, sin, gate, Wq, Wk, Wv, Wo
    )
    res = run_bass_kernel_spmd(nc, in_maps, core_ids=list(range(8)), **kwargs)
    acc = res.results[0]["outp"].astype(np.float32)
    for c in range(1, 8):
        acc = acc + res.results[c]["outp"].astype(np.float32)
    out = acc.reshape(1, S, D)
    if kwargs:
        return out, res
    return out


# revision 29
# speedup vs baseline: 1.0434x; 1.0434x over previous
"""GQA attention (16 q heads / 4 kv heads, HD=128, S=4096, D=2048) with RoPE,
causal mask, log-gate on kv positions, softmax, and output projection —
distributed over 8 NeuronCores.

Sharding: head-parallel. Core c computes q heads {2c, 2c+1} and kv head c//2.
Wq/Wk/Wv are split column-wise, Wo row-wise; each core produces a partial
[S, D] output (its 2 heads' contribution through Wo) and the host sums the 8
partials (the unshard step of the row-parallel Wo matmul).

On-device layout strategy (v3):
 - All matmul inputs fp16 (1 cycle/row like bf16, 8x the mantissa).
 - Projections computed transposed: qT/kT = W.T @ X.T with d on partitions.
 - Head-dim of q/k is PERMUTED (d and d+64 placed 16 apart within the same
   32-partition quadrant) so the RoPE rotate-half is a single DVE
   stream_shuffle straight out of PSUM — no SBUF round-trip, no DMA.
   Scores are invariant to any consistent d-permutation of q and k.
 - All DRAM inputs host-staged in exact SBUF tile layout: every DMA is 128
   large contiguous per-partition descriptors (the v2 kernel was
   descriptor-latency-bound with 41K small descriptors).
 - Attention computed transposed (scores^T [j, i]); max-free softmax with a
   constant -8 shift so fp16 exp/accumulate never overflows; log-gate +
   shift folded into the exp as a per-partition activation bias.
 - Causality structural: upper-triangle blocks skipped, fully-masked query
   columns of diagonal blocks skipped (ragged matmuls), the diagonal 128
   queries masked by a 0/1 fp16 multiply after exp.
 - Softmax denominators: fp16 DVE accumulation of exp tiles, one M=1
   ones-matmul per head, reciprocal_approx_fast, gpsimd partition-broadcast.
 - Emission pipelined: attn(nb) -> sums -> proj(nb+1) -> normalize(nb) ->
   outproj(nb); x(nb+1) prefetched an iteration ahead.
 - Per-core partial outputs written fp16 (halves write traffic; host sums
   in fp32).
"""

import math
from contextlib import ExitStack

import numpy as np

import concourse.bass as bass
import concourse.mybir as mybir
import concourse.tile as tile
from concourse import bacc
from concourse._compat import with_exitstack
from concourse.bass import ds
from concourse.bass_utils import run_bass_kernel_spmd
from concourse.masks import make_identity

P = 128
F = 512            # free-dim chunk (one PSUM bank of fp32)
S = 4096
D = 2048
HD = 128
KO = D // P        # 16 k-chunks for the projections
NB = S // F        # 8 sequence chunks
NJB = S // P       # 32 key blocks
F32 = mybir.dt.float32
FP16 = mybir.dt.float16
SHIFT = 8.0        # constant softmax shift (cancels in the ratio)
# quadrant-local swap of the 16-element halves: partition p <-> p +- 16
SHUF = list(range(16, 32)) + list(range(16))
# PERM[p] = head-dim stored at partition p (pairs d, d+64 live 16 apart)
PERM = np.array(
    [16 * q + s if s < 16 else 64 + 16 * q + (s - 16)
     for q in range(4) for s in range(32)]
)


@with_exitstack
def _body(ctx: ExitStack, tc: tile.TileContext, io: dict):
    nc = tc.nc

    persist = ctx.enter_context(tc.tile_pool(name="persist", bufs=1))
    qT = persist.tile([P, 2, S], FP16, tag="qT")        # [d, h, i]
    kT = persist.tile([P, S], FP16, tag="kT")           # [d, j]
    vv = persist.tile([P, NJB, HD], FP16, tag="vv")     # [j, jb, d]
    attnT = persist.tile([P, 2, S], FP16, tag="attnT")  # [d, h, i] normalized
    logg = persist.tile([P, NJB], F32, tag="logg")      # log(gate)-SHIFT, [j, jb]
    dmask01 = persist.tile([P, P], FP16, tag="dmask01")
    ident = persist.tile([P, P], F32, tag="ident")
    ones16 = persist.tile([P, 1], FP16, tag="ones16")

    wpool = ctx.enter_context(tc.tile_pool(name="wpool", bufs=1))
    wq = wpool.tile([P, KO, 2 * HD], FP16, tag="wq")
    wk = wpool.tile([P, KO, HD], FP16, tag="wk")
    wv = wpool.tile([P, KO, HD], FP16, tag="wv")
    wo = wpool.tile([P, 2, D], FP16, tag="wo")

    xt_pool = ctx.enter_context(tc.tile_pool(name="xt", bufs=2))
    tab_pool = ctx.enter_context(tc.tile_pool(name="tab", bufs=2))
    rope_pool = ctx.enter_context(tc.tile_pool(name="rope", bufs=2))
    exp_pool = ctx.enter_context(tc.tile_pool(name="exp", bufs=4))
    acc_pool = ctx.enter_context(tc.tile_pool(name="acc", bufs=2))
    bc_pool = ctx.enter_context(tc.tile_pool(name="bc", bufs=2))
    ob_pool = ctx.enter_context(tc.tile_pool(name="ob", bufs=3))
    # PSUM budget (8 banks): psSc pair tiles 2x2 + psAV 3 + psSum 1 = 8.
    psSc = ctx.enter_context(tc.tile_pool(name="psSc", bufs=2, space="PSUM"))
    psAV = ctx.enter_context(tc.tile_pool(name="psAV", bufs=3, space="PSUM"))
    psSum = ctx.enter_context(tc.tile_pool(name="psSum", bufs=1, space="PSUM"))

    def load_x(nb):
        xtile = xt_pool.tile([P, KO, F], FP16, tag="xt")
        nc.sync.dma_start(xtile[:], io["xs"][nb])
        tabs = tab_pool.tile([P, 4, F], FP16, tag="tabs")
        nc.sync.dma_start(tabs[:], io["tabs"][nb])
        return xtile, tabs

    # Startup: x + q-weights first so the first proj chain starts while the
    # remaining weights/constants stream in.
    xq0 = xt_pool.tile([P, KO, F], FP16, tag="xt", name="xq0")
    nc.sync.dma_start(xq0[:, 0:2, :], io["xs"][0, :, 0:2, :])
    nc.sync.dma_start(wq[:, 0:2, :], io["wq"][:, 0:2, :])
    nc.sync.dma_start(xq0[:, 2:8, :], io["xs"][0, :, 2:8, :])
    nc.sync.dma_start(wq[:, 2:8, :], io["wq"][:, 2:8, :])
    nc.sync.dma_start(xq0[:, 8:16, :], io["xs"][0, :, 8:16, :])
    nc.sync.dma_start(wq[:, 8:16, :], io["wq"][:, 8:16, :])
    tabs0 = tab_pool.tile([P, 4, F], FP16, tag="tabs", name="tabs0")
    nc.sync.dma_start(tabs0[:], io["tabs"][0])
    nc.sync.dma_start(wk[:], io["wk"])
    nc.sync.dma_start(wv[:], io["wv"])
    nc.sync.dma_start(logg[:], io["logg"])
    nc.sync.dma_start(dmask01[:], io["dmask01"])
    nc.sync.dma_start(wo[:], io["wo"])
    make_identity(nc, ident[:])
    nc.vector.memset(ones16[:], 1.0)

    def rope(ps, ct, st, dest, pair=True):
        src_ap = ps[:, 0, :] if pair else ps[:]
        rot = rope_pool.tile([P, F], F32, tag="rot", name="rot")
        nc.vector.stream_shuffle(rot[:], src_ap, mask=SHUF)
        t1 = rope_pool.tile([P, F], F32, tag="t1", name="t1")
        nc.vector.tensor_tensor(t1[:], src_ap, ct, op=mybir.AluOpType.mult)
        r2 = rope_pool.tile([P, F], F32, tag="r2", name="r2")
        nc.vector.tensor_tensor(r2[:], rot[:], st, op=mybir.AluOpType.mult)
        nc.vector.tensor_tensor(dest, t1[:], r2[:], op=mybir.AluOpType.add)

    def vchain_work(nb, xtile):
        """Closure-pair for the deferred V chain of chunk nb: step(i) emits
        the i-th slice of [16 MMs, vT copy, 4 transposes, vv copy] so
        outproj can interleave them into its evac-stall slots."""
        state = {"ps": None, "vT": None, "pt": None, "done": 0}

        def step():
            i = state["done"]
            if i < KO:
                if i == 0:
                    state["ps"] = psSc.tile([P, 2, F], F32, tag="sc",
                                            name="vps")
                nc.tensor.matmul(
                    state["ps"][:, 0, :],
                    lhsT=wv[:, i, :],
                    rhs=xtile[:, i, :],
                    start=(i == 0),
                    stop=(i == KO - 1),
                )
            elif i == KO:
                state["vT"] = rope_pool.tile([P, F], F32, tag="vT",
                                             name="vT")
                nc.scalar.copy(state["vT"][:], state["ps"][:, 0, :])
            elif i <= KO + 4:
                isub = i - KO - 1
                if isub == 0:
                    state["pt"] = psSc.tile([P, 2, F], F32, tag="sc",
                                            name="vpt")
                nc.tensor.transpose(
                    state["pt"][:, 0, ds(isub * P, P)],
                    state["vT"][:, ds(isub * P, P)], ident[:],
                )
            elif i == KO + 5:
                nc.scalar.copy(vv[:, ds(nb * 4, 4), :], state["pt"][:, 0, :])
            state["done"] = i + 1

        return step, KO + 6

    def proj(nb, xtile, tabs, with_k=True, with_v=True, mid=None):
        """Projections + rope + v-transpose for sequence chunk nb. `mid` is
        emitted after the first chain so the tensor engine has ready work
        (the Q0 chain) to absorb the wait for the attn accumulator."""
        sl = ds(nb * F, F)

        def chain(w_sb, m0):
            ps = psSc.tile([P, 2, F], F32, tag="sc")
            for ko in range(KO):
                nc.tensor.matmul(
                    ps[:, 0, :],
                    lhsT=w_sb[:, ko, ds(m0, P)],
                    rhs=xtile[:, ko, :],
                    start=(ko == 0),
                    stop=(ko == KO - 1),
                )
            return ps

        rope(chain(wq, 0), tabs[:, 0, :], tabs[:, 1, :], qT[:, 0, sl])
        mids = [mid() if mid is not None else None]
        rope(chain(wq, P), tabs[:, 0, :], tabs[:, 1, :], qT[:, 1, sl])
        if with_k:
            rope(chain(wk, 0), tabs[:, 2, :], tabs[:, 3, :], kT[:, sl])

        if with_v:
            vstep, vn = vchain_work(nb, xtile)
            for _ in range(vn):
                vstep()

    def attn(nb, kchain=None):
        """Attention for q-chunk nb (both heads); returns psum AV tiles +
        the fp16 exp-accumulator (for the sum collapse). If kchain is
        (xtile, tabs, nbn), the K projection chain for chunk nbn is
        interleaved into the jb loop (fills the tensor engine's slack in
        the scalar-paced attention window, psSum bank is free here)."""
        njb = 4 * nb + 4
        avs = [
            psAV.tile([P, F], F32, tag="av", name=f"av{h}") for h in range(2)
        ]
        acc = acc_pool.tile([P, 2, F], FP16, tag="acc")
        ck_done = 0
        if kchain is not None:
            xtn, tabn, nbn = kchain
            ck = psSum.tile([P, F], F32, tag="cksum", name="ck")

            def ck_emit(limit):
                nonlocal ck_done
                while ck_done < min(limit, KO):
                    nc.tensor.matmul(
                        ck[:],
                        lhsT=wk[:, ck_done, :],
                        rhs=xtn[:, ck_done, :],
                        start=(ck_done == 0),
                        stop=(ck_done == KO - 1),
                    )
                    ck_done += 1


        for jb in range(njb):
            dp = jb - 4 * nb
            # Queries below the diagonal block are fully masked: skip them.
            q0 = dp * P if dp > 0 else 0
            n = F - q0
            sc = psSc.tile([P, 2, F], F32, tag="sc")
            for h in range(2):
                nc.tensor.matmul(
                    sc[:, h, 0:n],
                    lhsT=kT[:, ds(jb * P, P)],
                    rhs=qT[:, h, ds(nb * F + q0, n)],
                    start=True,
                    stop=True,
                )
            ex = exp_pool.tile([P, 2, F], FP16, tag="ex")
            nc.scalar.activation(
                ex[:, :, 0:n], sc[:, :, 0:n],
                mybir.ActivationFunctionType.Exp,
                bias=logg[:, jb : jb + 1],
            )
            if dp >= 0:
                # within-block triangle mask on the diagonal 128 queries
                for h in range(2):
                    nc.vector.tensor_tensor(
                        ex[:, h, 0:P], ex[:, h, 0:P], dmask01[:],
                        op=mybir.AluOpType.mult,
                    )
            if jb == 0:
                nc.vector.tensor_copy(acc[:], ex[:])
            else:
                nc.vector.tensor_tensor(
                    acc[:, :, q0:F], acc[:, :, q0:F], ex[:, :, 0:n],
                    op=mybir.AluOpType.add,
                )
            for h in range(2):
                nc.tensor.matmul(
                    avs[h][:, q0:F],
                    lhsT=vv[:, jb, :],
                    rhs=ex[:, h, 0:n],
                    start=(jb == 0),
                    stop=(jb == njb - 1),
                )
            if kchain is not None and jb >= 6:
                # uniform spread of the 16 chain MMs over the eligible jb
                ck_emit((jb - 5) * KO // max(njb - 6, 1))
        if kchain is not None:
            ck_emit(KO)
            rope(ck, tabn[:, 2, :], tabn[:, 3, :], kT[:, ds(nbn * F, F)],
                 pair=False)
        return avs, acc

    def sums_collapse(acc):
        sums = psSum.tile([P, F], F32, tag="cksum", name="sums")
        for h in range(2):
            nc.tensor.matmul(
                sums[ds(32 * h, 1), :],
                lhsT=ones16[:, 0:1],
                rhs=acc[:, h, :],
                start=True,
                stop=True,
            )
        return sums

    def normalize(nb, avs, sums):
        sl = ds(nb * F, F)
        for h in range(2):
            srow = bc_pool.tile([1, F], F32, tag=f"srow{h}", name=f"srow{h}")
            nc.scalar.copy(srow[:], sums[ds(32 * h, 1), :])
            rrow = bc_pool.tile([1, F], F32, tag=f"rrow{h}", name=f"rrow{h}")
            nc.vector.reciprocal_approx_fast(rrow[:], srow[:])
            rbc = bc_pool.tile([P, F], F32, tag=f"rbc{h}", name=f"rbc{h}")
            nc.gpsimd.partition_broadcast(rbc[:], rrow[0:1, :])
            nc.vector.tensor_tensor(
                attnT[:, h, sl], avs[h][:], rbc[:],
                op=mybir.AluOpType.mult,
            )

    def outproj(nb, last=False, vwork=None):
        vstep, vn, vdone = None, 0, 0
        if vwork is not None:
            vstep, vn = vwork
        for i4 in range(4):
            i2 = nb * 4 + i4
            split = last and i4 == 3
            ob = ob_pool.tile([P, D], FP16, tag="ob")
            for e in range(D // F):
                po = psAV.tile([P, F], F32, tag="av")
                for h in range(2):
                    nc.tensor.matmul(
                        po[:],
                        lhsT=attnT[:, h, ds(i2 * P, P)],
                        rhs=wo[:, h, ds(e * F, F)],
                        start=(h == 0),
                        stop=(h == 1),
                    )
                # fill the evac-gated stall slots with the next chunk's
                # V-projection work
                target = min(vn, ((i4 * 4 + e + 1) * vn) // 14)
                while vdone < target:
                    vstep()
                    vdone += 1
                if e % 2 == 0:
                    nc.scalar.copy(ob[:, ds(e * F, F)], po[:])
                else:
                    nc.vector.tensor_copy(ob[:, ds(e * F, F)], po[:])
                if split:
                    nc.sync.dma_start(
                        io["outp"][ds(i2 * P, P), ds(e * F, F)],
                        ob[:, ds(e * F, F)],
                    )
            if not split:
                nc.sync.dma_start(io["outp"][ds(i2 * P, P), :], ob[:])

    # ---- pipelined emission ----
    # load_x(nb+2) is issued AFTER outproj(nb)'s output DMAs so the output
    # writes are never queued behind the multi-MB prefetch (head-of-line
    # blocking on the DMA queues was stalling psum-tile reuse).
    proj(0, xq0, tabs0)
    xqn, tabsn = load_x(1)
    for nb in range(NB):
        kc = (xqn, tabsn, nb + 1) if nb + 1 < NB else None
        avs, acc = attn(nb, kchain=kc)
        vw = None
        if nb + 1 < NB:
            xq_cur, tabs_cur = xqn, tabsn
            holder = []
            proj(nb + 1, xq_cur, tabs_cur, with_k=False, with_v=False,
                 mid=lambda: holder.append(sums_collapse(acc)))
            sums = holder[0]
            vw = vchain_work(nb + 1, xq_cur)
        else:
            sums = sums_collapse(acc)
        normalize(nb, avs, sums)
        outproj(nb, last=(nb == NB - 1), vwork=vw)
        if nb + 2 < NB:
            xqn, tabsn = load_x(nb + 2)


_NC_CACHE = None


def build_nc():
    global _NC_CACHE
    if _NC_CACHE is not None:
        return _NC_CACHE
    nc = bacc.Bacc("TRN2", target_bir_lowering=False, debug=False)
    io = {
        "xs": nc.dram_tensor("xs", [NB, P, KO, F], FP16,
                             kind="ExternalInput").ap(),
        "wq": nc.dram_tensor("wq", [P, KO, 2 * HD], FP16,
                             kind="ExternalInput").ap(),
        "wk": nc.dram_tensor("wk", [P, KO, HD], FP16,
                             kind="ExternalInput").ap(),
        "wv": nc.dram_tensor("wv", [P, KO, HD], FP16,
                             kind="ExternalInput").ap(),
        "wo": nc.dram_tensor("wo", [P, 2, D], FP16,
                             kind="ExternalInput").ap(),
        "tabs": nc.dram_tensor("tabs", [NB, P, 4, F], FP16,
                               kind="ExternalInput").ap(),
        "logg": nc.dram_tensor("logg", [P, NJB], F32,
                               kind="ExternalInput").ap(),
        "dmask01": nc.dram_tensor("dmask01", [P, P], FP16,
                                  kind="ExternalInput").ap(),
        "outp": nc.dram_tensor("outp", [S, D], FP16,
                               kind="ExternalOutput").ap(),
    }
    with tile.TileContext(nc) as tc:
        _body(tc, io)
    nc.compile()
    _NC_CACHE = nc
    return nc


def make_in_maps(hidden_states, attention_mask, cos, sin, gate, Wq, Wk, Wv, Wo):
    X = np.asarray(hidden_states, np.float32).reshape(S, D)
    # [nb, p, ko, s] = X[nb*F + s, ko*P + p]
    xs = np.ascontiguousarray(
        X.reshape(NB, F, KO, P).transpose(0, 3, 2, 1).astype(np.float16)
    )
    cosT = np.asarray(cos, np.float32).reshape(S, HD).T      # [d, s]
    sinT = np.asarray(sin, np.float32).reshape(S, HD).T
    cosP = cosT[PERM]
    sign = np.where(np.arange(P) % 32 < 16, np.float32(-1), np.float32(1))
    sinP = sign[:, None] * sinT[PERM]
    sc = np.float32(1.0 / math.sqrt(HD))
    tab = np.stack([cosP * sc, sinP * sc, cosP, sinP], axis=1)  # [128, 4, S]
    tabs = np.ascontiguousarray(
        tab.reshape(P, 4, NB, F).transpose(2, 0, 1, 3).astype(np.float16)
    )
    g = np.asarray(gate, np.float32).reshape(S) + np.float32(1e-8)
    logg = np.log(g).astype(np.float32) - np.float32(SHIFT)
    logg = np.ascontiguousarray(logg.reshape(NJB, P).T)
    jj = np.arange(P)[:, None]
    ii = np.arange(P)[None, :]
    dmask01 = np.ascontiguousarray(
        np.where(jj <= ii, 1, 0).astype(np.float16)
    )

    Wq = np.asarray(Wq, np.float32)
    Wk = np.asarray(Wk, np.float32)
    Wv = np.asarray(Wv, np.float32)
    Wo = np.asarray(Wo, np.float32)

    def stage_w(Wc, nheads, perm):
        # Wc [D, nheads*128] -> [p, ko, m] fp16 (m within-head order perm'd)
        if perm is not None:
            Wc = Wc.reshape(D, nheads, P)[:, :, perm].reshape(D, nheads * P)
        return np.ascontiguousarray(
            Wc.reshape(KO, P, nheads * P).transpose(1, 0, 2).astype(np.float16)
        )

    in_maps = []
    for c in range(8):
        g128 = c // 2
        wo_c = Wo[c * 256 : (c + 1) * 256, :]
        in_maps.append(
            {
                "xs": xs,
                "wq": stage_w(Wq[:, c * 256 : (c + 1) * 256], 2, PERM),
                "wk": stage_w(Wk[:, g128 * HD : (g128 + 1) * HD], 1, PERM),
                "wv": stage_w(Wv[:, g128 * HD : (g128 + 1) * HD], 1, None),
                "wo": np.ascontiguousarray(
                    wo_c.reshape(2, P, D).transpose(1, 0, 2).astype(np.float16)
                ),
                "tabs": tabs,
                "logg": logg,
                "dmask01": dmask01,
            }
        )
    return in_maps


def kernel(hidden_states, attention_mask, cos, sin, gate, Wq, Wk, Wv, Wo,
           **kwargs):
    nc = build_nc()
    in_maps = make_in_maps(
        hidden_states, attention_mask, cos, sin, gate, Wq, Wk, Wv, Wo
    )
    res = run_bass_kernel_spmd(nc, in_maps, core_ids=list(range(8)), **kwargs)
    acc = res.results[0]["outp"].astype(np.float32)
    for c in range(1, 8):
        acc = acc + res.results[c]["outp"].astype(np.float32)
    out = acc.reshape(1, S, D)
    if kwargs:
        return out, res
    return out
